# revision 9
# baseline (speedup 1.0000x reference)
"""Top-k (64) causal multi-head attention for Trainium2, 8 NeuronCores.

Sharding: core c handles batch c//4 and heads (c%4)*4..+4. Each core computes
its heads' attention and a partial O-projection; the host sums per-batch
partials and adds the constant row (Wo@bv + bo).

Math:
 - scores = (q+bq)(k+bk)/8. Per-row-constant shifts cancel in top-k and
   softmax, so s ~ q.k + d_j with q = Wq x, k = Wk x (biasless) and
   d = (bq^T Wk) x folded in as rank-1 (K=1) aug matmuls.
 - Score matmuls run as 3-term fp16 splits (hi*hi + hi*lo + lo*hi): measured
   8.7e-7 rel err - top-64 selection matches the fp32 reference.
 - Exact top-64 per row: per-32-chunk max8 candidates, then 8 rounds of
   max8 + match_replace8. Threshold -> transposed broadcast -> S^T mask via a
   custom DVE select op -> exp (scalar engine) -> P^T @ [V|1] gives numerator
   and softmax denominator in one accumulation.
"""

import sys

sys.path.insert(0, "/opt/trn_rl_repo")

import ml_dtypes
import numpy as np

import concourse.bacc as bacc
import concourse.mybir as mybir
from concourse import dve_ops
from concourse.bass_utils import run_bass_kernel_spmd
from concourse.dve_spec import C0, Spec, Src0, Src1, select
from concourse.tile import TileContext

F32 = mybir.dt.float32
F16 = mybir.dt.float16
BF16 = mybir.dt.bfloat16
BF16_NP = ml_dtypes.bfloat16

B, S, D, H, DK = 2, 2048, 1024, 16, 64
NT = S // 128
NHC = 4
NCORES = 8
NEG = -1e30
XS = 64.0          # x' = 64 x,  w' = 64 W  -> scores' = 2^24 * q.k
WDS = float(2**10)  # d-column weight scale: d-psum = 2^16 * (w_d . x)
AUGV = 256.0        # K=1 aug value: 2^8 * 2^16 = 2^24 (same scale as q.k)
EXP_SCALE = float(2**-27)  # s_true = q.k/8 = scores' * 2^-27


def _register_masklt():
    """out = select(in0 < in1, c0, in0) - mask below a per-element threshold."""
    name = "MASKLT_ANT"
    for o in dve_ops.OPS:
        if o.name == name:
            return o
    spec = Spec(
        body=select(Src0 < Src1, C0, Src0),
        reference=lambda in0, in1, s0, s1, imm2: np.where(
            in0 < in1, np.float32(s0), in0
        ).astype(np.float32),
    )
    row = max(dve_ops._SUB_OPCODE_FOR_NAME.values()) + 1
    assert row < 0x20
    dve_ops._SUB_OPCODE_FOR_NAME[name] = row
    op = dve_ops.DveOp(name, spec, subdim=False, uops_sha={})
    sha = {}
    for ver in ("v3",):
        try:
            op.compile(ver)
        except ValueError as e:
            import re

            sha[ver] = re.search(rf"{ver}: ([0-9a-f]+) ", str(e)).group(1)
    op = dve_ops.DveOp(name, spec, subdim=False, uops_sha=sha)
    dve_ops.OPS.append(op)
    dve_ops.CUSTOM_DVE_SPECS[name] = spec
    return op


def _build_nc():
    MASKLT = _register_masklt()
    nc = bacc.Bacc()

    def din(name, shape, dt):
        return nc.dram_tensor(name, shape, dt, kind="ExternalInput")

    xqh_d = din("xqh", [D, S], F16)
    xql_d = din("xql", [D, S], F16)
    xkh_d = din("xkh", [D, S], F16)
    xkl_d = din("xkl", [D, S], F16)
    xv_d = din("xv", [D, S], F16)
    wqh_d = din("wqh", [D, 256], F16)
    wql_d = din("wql", [D, 256], F16)
    wkh_d = din("wkh", [D, 260], F16)
    wkl_d = din("wkl", [D, 260], F16)
    wv_d = din("wv", [D, 256], F16)
    wo_d = din("wo", [256, 1024], F16)
    cm_d = din("cm", [128, 128], F32)
    cmT_d = din("cmT", [128, 128], F32)
    fo = nc.dram_tensor("fo", [D, S], F32, kind="ExternalOutput")

    with TileContext(nc) as tc:
        with tc.tile_pool(name="res", bufs=1) as rp:
            wqh = rp.tile([128, 8, 256], F16, tag="wqh")
            wql = rp.tile([128, 8, 256], F16, tag="wql")
            wkh = rp.tile([128, 8, 260], F16, tag="wkh")
            wkl = rp.tile([128, 8, 260], F16, tag="wkl")
            wv = rp.tile([128, 8, 256], F16, tag="wv")
            wo = rp.tile([128, 2, 1024], F16, tag="wo")
            cm = rp.tile([128, 128], F32, tag="cm")
            cmT = rp.tile([128, 128], F32, tag="cmT")
            for kc in range(8):
                sl = slice(kc * 128, kc * 128 + 128)
                nc.sync.dma_start(wqh[:, kc], wqh_d[sl, :])
                nc.sync.dma_start(wql[:, kc], wql_d[sl, :])
                nc.sync.dma_start(wkh[:, kc], wkh_d[sl, :])
                nc.sync.dma_start(wkl[:, kc], wkl_d[sl, :])
                nc.sync.dma_start(wv[:, kc], wv_d[sl, :])
            for p in range(2):
                nc.sync.dma_start(wo[:, p], wo_d[p * 128 : p * 128 + 128, :])
            nc.sync.dma_start(cm[:], cm_d[:])
            nc.sync.dma_start(cmT[:], cmT_d[:])

            # projected tensors (pair-packed: head 2p at rows 0..63, 2p+1 at 64..127)
            qh = rp.tile([128, 2, S], F16, tag="qh")
            ql = rp.tile([128, 2, S], F16, tag="ql")
            kh = rp.tile([128, 2, S], F16, tag="kh")
            kl = rp.tile([128, 2, S], F16, tag="kl")
            dht = [rp.tile([1, S], F16, tag=f"dh{hh}", name=f"dh{hh}") for hh in range(4)]
            dlt = [rp.tile([1, S], F16, tag=f"dl{hh}", name=f"dl{hh}") for hh in range(4)]
            vt = rp.tile([128, NT, 4, 65], F16, tag="vt")
            obuf = rp.tile([128, 2, S], F16, tag="obuf")
            onesq = rp.tile([1, 128], F16, tag="onesq")
            nc.vector.memset(onesq[:], AUGV)
            nc.vector.memset(vt[:, :, :, 64:65], 1.0)

            # ---------------- projections ----------------
            with tc.tile_pool(name="xin", bufs=2) as xp, tc.tile_pool(
                name="pjp", bufs=2, space="PSUM"
            ) as pj:
                for tsr, xh_d, xl_d, w_h, w_l, dsth, dstl in (
                    ("q", xqh_d, xql_d, wqh, wql, qh, ql),
                    ("k", xkh_d, xkl_d, wkh, wkl, kh, kl),
                ):
                    for nt in range(4):
                        nsl = slice(nt * 512, nt * 512 + 512)
                        xhs, xls = [], []
                        for kc in range(8):
                            th = xp.tile([128, 512], F16, tag=f"xh{kc}")
                            tl = xp.tile([128, 512], F16, tag=f"xl{kc}")
                            ksl = slice(kc * 128, kc * 128 + 128)
                            nc.sync.dma_start(th[:], xh_d[ksl, nsl])
                            nc.sync.dma_start(tl[:], xl_d[ksl, nsl])
                            xhs.append(th)
                            xls.append(tl)
                        for p in range(2):
                            psl = slice(p * 128, p * 128 + 128)
                            ps = pj.tile([128, 512], F32, tag="pj")
                            for kc in range(8):
                                nc.tensor.matmul(
                                    ps[:], w_h[:, kc, psl], xhs[kc][:],
                                    start=(kc == 0), stop=False,
                                )
                            for kc in range(8):
                                nc.tensor.matmul(
                                    ps[:], w_l[:, kc, psl], xhs[kc][:],
                                    start=False, stop=False,
                                )
                            for kc in range(8):
                                nc.tensor.matmul(
                                    ps[:], w_h[:, kc, psl], xls[kc][:],
                                    start=False, stop=(kc == 7),
                                )
                            nc.scalar.copy(dsth[:, p, nsl], ps[:])
                            nc.vector.tensor_sub(dstl[:, p, nsl], ps[:], dsth[:, p, nsl])
                        if tsr == "k":
                            ps = pj.tile([128, 512], F32, tag="pjd")
                            for kc in range(8):
                                nc.tensor.matmul(
                                    ps[0:4, :], w_h[:, kc, 256:260], xhs[kc][:],
                                    start=(kc == 0), stop=False,
                                )
                            for kc in range(8):
                                nc.tensor.matmul(
                                    ps[0:4, :], w_l[:, kc, 256:260], xhs[kc][:],
                                    start=False, stop=False,
                                )
                            for kc in range(8):
                                nc.tensor.matmul(
                                    ps[0:4, :], w_h[:, kc, 256:260], xls[kc][:],
                                    start=False, stop=(kc == 7),
                                )
                            dth = xp.tile([4, 512], F16, tag="dth")
                            dtl = xp.tile([4, 512], F16, tag="dtl")
                            nc.scalar.copy(dth[:], ps[0:4, :])
                            nc.vector.tensor_sub(dtl[:], ps[0:4, :], dth[:])
                            for h in range(4):
                                nc.sync.dma_start(dht[h][:, nsl], dth[h : h + 1, :])
                                nc.sync.dma_start(dlt[h][:, nsl], dtl[h : h + 1, :])
                for t16 in range(NT):
                    tsl = slice(t16 * 128, t16 * 128 + 128)
                    xcs = []
                    for kc in range(8):
                        tv = xp.tile([128, 128], F16, tag=f"xv{kc}")
                        nc.sync.dma_start(tv[:], xv_d[kc * 128 : kc * 128 + 128, tsl])
                        xcs.append(tv)
                    ps = pj.tile([128, 512], F32, tag="pjv")
                    for kc in range(8):
                        nc.tensor.matmul(
                            ps[:, 0:256], xcs[kc][:], wv[:, kc],
                            start=(kc == 0), stop=(kc == 7),
                        )
                    nc.scalar.copy(
                        vt[:, t16, :, 0:64],
                        ps[:, 0:256].rearrange("p (h d) -> p h d", h=4),
                    )

            # ---------------- attention ----------------
            with tc.tile_pool(name="atw", bufs=2) as wp, tc.tile_pool(
                name="psS", bufs=1, space="PSUM"
            ) as ppS, tc.tile_pool(name="psT", bufs=2, space="PSUM") as ppT, tc.tile_pool(
                name="psO", bufs=2, space="PSUM"
            ) as ppO:
                for h in range(4):
                    p, hp = h // 2, h % 2
                    rsl = slice(hp * 64, hp * 64 + 64)
                    for i in range(NT):
                        W = (i + 1) * 128
                        isl = slice(i * 128, i * 128 + 128)
                        # --- S [128q, W]
                        psS = ppS.tile([128, 2048], F32, tag="S")
                        for ch in range((W + 511) // 512):
                            csl = slice(ch * 512, min(W, ch * 512 + 512))
                            nc.tensor.matmul(
                                psS[:, csl], qh[rsl, p, isl], kh[rsl, p, csl],
                                start=True, stop=False,
                            )
                            nc.tensor.matmul(
                                psS[:, csl], ql[rsl, p, isl], kh[rsl, p, csl],
                                start=False, stop=False,
                            )
                            nc.tensor.matmul(
                                psS[:, csl], qh[rsl, p, isl], kl[rsl, p, csl],
                                start=False, stop=False,
                            )
                            nc.tensor.matmul(
                                psS[:, csl], onesq[:], dht[h][:, csl],
                                start=False, stop=False,
                            )
                            nc.tensor.matmul(
                                psS[:, csl], onesq[:], dlt[h][:, csl],
                                start=False, stop=True,
                            )
                        nc.vector.tensor_add(psS[:, isl], psS[:, isl], cm[:])
                        stS = wp.tile([128, 2048], F32, tag="stS")
                        for ch in range((W + 511) // 512):
                            csl = slice(ch * 512, min(W, ch * 512 + 512))
                            nc.scalar.copy(stS[:, csl], psS[:, csl])
                        # --- exact top-64 -> mx
                        mx = wp.tile([128, 64], F32, tag="mx")
                        if W <= 1024:
                            cview = stS[:, 0:W]
                        else:
                            csz = W // 64
                            cands = wp.tile([128, 512], F32, tag="cands")
                            for cc in range(64):
                                nc.vector.max(
                                    cands[:, cc * 8 : cc * 8 + 8],
                                    stS[:, cc * csz : cc * csz + csz],
                                )
                            cview = cands[:]
                        for r in range(8):
                            nc.vector.max(mx[:, r * 8 : r * 8 + 8], cview)
                            if r < 7:
                                nc.vector.match_replace(
                                    out=cview, in_to_replace=mx[:, r * 8 : r * 8 + 8],
                                    in_values=cview, imm_value=NEG,
                                )
                        # --- threshold -> tb4 [128, 512]
                        tcol = wp.tile([128, 32], F32, tag="tcol")
                        nc.vector.memset(tcol[:], 0.0)
                        nc.vector.tensor_copy(tcol[:, 0:1], mx[:, 63:64])
                        ttr = wp.tile([128, 32], F32, tag="ttr")
                        nc.vector.transpose(ttr[:], tcol[:])
                        tvec = wp.tile([1, 512], F32, tag="tvec")
                        for bb in range(4):
                            nc.vector.tensor_copy(
                                tvec[:, bb * 32 : bb * 32 + 32],
                                ttr[bb * 32 : bb * 32 + 1, 0:32],
                            )
                        for kk in range(1, 4):
                            nc.vector.tensor_copy(
                                tvec[:, kk * 128 : kk * 128 + 128], tvec[:, 0:128]
                            )
                        tb4 = wp.tile([128, 512], F32, tag="tb4")
                        nc.gpsimd.partition_broadcast(tb4[:], tvec[:])
                        # --- S^T blocks -> mask -> exp -> P^T
                        nblk = i + 1
                        mstg = wp.tile([128, NT, 128], F32, tag="mstg")
                        for jg in range((nblk + 3) // 4):
                            jn = min(nblk, jg * 4 + 4) - jg * 4
                            psT = ppT.tile([128, 512], F32, tag="T")
                            for jj in range(jn):
                                j = jg * 4 + jj
                                jsl = slice(j * 128, j * 128 + 128)
                                osl = slice(jj * 128, jj * 128 + 128)
                                nc.tensor.matmul(
                                    psT[:, osl], kh[rsl, p, jsl], qh[rsl, p, isl],
                                    start=True, stop=False,
                                )
                                nc.tensor.matmul(
                                    psT[:, osl], kh[rsl, p, jsl], ql[rsl, p, isl],
                                    start=False, stop=False,
                                )
                                nc.tensor.matmul(
                                    psT[:, osl], kl[rsl, p, jsl], qh[rsl, p, isl],
                                    start=False, stop=False,
                                )
                                nc.tensor.matmul(
                                    psT[:, osl], dht[h][:, jsl], onesq[:],
                                    start=False, stop=False,
                                )
                                nc.tensor.matmul(
                                    psT[:, osl], dlt[h][:, jsl], onesq[:],
                                    start=False, stop=True,
                                )
                                if j == i:
                                    nc.vector.tensor_add(
                                        psT[:, osl], psT[:, osl], cmT[:]
                                    )
                            nc.vector._custom_dve(
                                MASKLT,
                                out=mstg[:, jg * 4 : jg * 4 + jn, :],
                                in0=psT[:, 0 : jn * 128].rearrange(
                                    "p (a b) -> p a b", b=128
                                ),
                                in1=tb4[:, 0 : jn * 128].rearrange(
                                    "p (a b) -> p a b", b=128
                                ),
                                s0=NEG,
                            )
                        pt = wp.tile([128, NT, 128], F16, tag="pt")
                        nc.scalar.activation(
                            pt[:, 0:nblk, :], mstg[:, 0:nblk, :],
                            mybir.ActivationFunctionType.Exp, scale=EXP_SCALE,
                        )
                        # --- P^T @ [V|1]
                        psO = ppO.tile([128, 128], F32, tag="O")
                        for j in range(nblk):
                            nc.tensor.matmul(
                                psO[0:65, :], vt[:, j, h, :], pt[:, j, :],
                                start=(j == 0), stop=(j == nblk - 1),
                            )
                        # --- normalize -> obuf
                        rr = wp.tile([1, 128], F32, tag="rr")
                        nc.vector.reciprocal(rr[:], psO[64:65, :])
                        rb = wp.tile([128, 128], F32, tag="rb")
                        nc.gpsimd.partition_broadcast(rb[0:64, :], rr[:])
                        if hp == 0:
                            nc.vector.tensor_mul(
                                obuf[0:64, p, isl], psO[0:64, :], rb[0:64, :]
                            )
                        else:
                            otmp = wp.tile([64, 128], F16, tag="otmp")
                            nc.vector.tensor_mul(otmp[:], psO[0:64, :], rb[0:64, :])
                            nc.sync.dma_start(obuf[64:128, p, isl], otmp[:])

            # ---------------- O-projection ----------------
            with tc.tile_pool(name="psF", bufs=2, space="PSUM") as ppF, tc.tile_pool(
                name="fout", bufs=3
            ) as fp:
                for dt in range(8):
                    dsl = slice(dt * 128, dt * 128 + 128)
                    for ntc in range(4):
                        nsl = slice(ntc * 512, ntc * 512 + 512)
                        psF = ppF.tile([128, 512], F32, tag="F")
                        for p in range(2):
                            nc.tensor.matmul(
                                psF[:], wo[:, p, dsl], obuf[:, p, nsl],
                                start=(p == 0), stop=(p == 1),
                            )
                        fstg = fp.tile([128, 512], F32, tag="fstg")
                        nc.scalar.copy(fstg[:], psF[:])
                        nc.sync.dma_start(fo[dsl, nsl], fstg[:])

    nc.compile()
    return nc


_NC_CACHE = None


def _get_nc():
    global _NC_CACHE
    if _NC_CACHE is None:
        _NC_CACHE = _build_nc()
    return _NC_CACHE


def _host_prep(inputs):
    q_in, k_in, v_in = inputs["query"], inputs["key"], inputs["value"]
    Wq, Wk = inputs["Wq"], inputs["Wk"]
    bq = inputs["bq"]
    f32 = np.float32
    cm = np.where(
        np.arange(128)[None, :] > np.arange(128)[:, None], f32(NEG), f32(0)
    ).astype(f32)
    cmT = np.ascontiguousarray(cm.T)

    def split16(a):
        hi = a.astype(np.float16)
        lo = (a - hi.astype(f32)).astype(np.float16)
        return hi, lo

    xq_s, xk_s, xv_s = {}, {}, {}
    for b in range(B):
        xq_s[b] = split16(np.ascontiguousarray(q_in[b].T) * f32(XS))
        xk_s[b] = split16(np.ascontiguousarray(k_in[b].T) * f32(XS))
        xv_s[b] = np.ascontiguousarray(v_in[b].T).astype(np.float16)

    in_maps = []
    for c in range(NCORES):
        b = c // 4
        h0 = (c % 4) * NHC
        dsl = slice(h0 * DK, h0 * DK + NHC * DK)
        wq64 = np.ascontiguousarray(Wq[dsl, :].T) * f32(XS)
        wk64 = np.ascontiguousarray(Wk[dsl, :].T) * f32(XS)
        dcols = np.zeros((D, NHC), dtype=f32)
        for hh in range(NHC):
            hsl = slice((h0 + hh) * DK, (h0 + hh) * DK + DK)
            dcols[:, hh] = (Wk[hsl, :].T @ bq[hsl]) * f32(WDS)
        wk_aug = np.concatenate([wk64, dcols], axis=1)
        wqh_, wql_ = split16(wq64)
        wkh_, wkl_ = split16(wk_aug)
        in_maps.append(
            {
                "xqh": xq_s[b][0], "xql": xq_s[b][1],
                "xkh": xk_s[b][0], "xkl": xk_s[b][1],
                "xv": xv_s[b],
                "wqh": wqh_, "wql": wql_, "wkh": wkh_, "wkl": wkl_,
                "wv": np.ascontiguousarray(inputs["Wv"][dsl, :].T).astype(np.float16),
                "wo": np.ascontiguousarray(inputs["Wo"][:, dsl].T).astype(np.float16),
                "cm": cm, "cmT": cmT,
            }
        )
    return in_maps


def _host_post(results, inputs):
    acc = np.zeros((B, D, S), dtype=np.float64)
    for c, r in enumerate(results):
        acc[c // 4] += r["fo"].astype(np.float64)
    out = np.ascontiguousarray(acc.transpose(0, 2, 1)).astype(np.float32)
    bias_row = (inputs["Wo"] @ inputs["bv"] + inputs["bo"]).astype(np.float32)
    out += bias_row[None, None, :]
    return out


def kernel(**inputs) -> np.ndarray:
    nc = _get_nc()
    in_maps = _host_prep(inputs)
    res = run_bass_kernel_spmd(nc, in_maps, core_ids=list(range(NCORES)))
    return _host_post(res.results, inputs)
